# revision 9
# baseline (speedup 1.0000x reference)
"""JointRetention Trainium2 kernel.

out[b] = ((xpos(X_b Wq) xpos_down(X_b Wk)^T) * D[b%17]) @ (X_b Wv)

Strategy (v3):
  - Data-parallel over B*J=1088 across 8 cores (136 each; 136%17==0 so the
    joint index pattern is identical on every core).
  - Even/odd d-permutation: with head-dim columns reordered [even | odd],
    rotate_every_two becomes a half-swap, so xpos needs NO extra projection:
      Qx_e = Qe*hC - Qo*hS ;  Qx_o = Qo*hC + Qe*hS
  - X stored as bf16 and loaded TRANSPOSED via the XBAR DMA-transpose
    (h lands on partitions directly): no PE transposes, no psum round trip.
  - All matmul operands bf16 (f32 PSUM): 1 cycle/col at any free size.
  - Block-causal sparsity: scores for m in [128,243) computed only for
    l >= 81 (decay mask zeroes the rest; the at-tile columns for l<81 are
    psum-garbage * 0 table = 0).
  - PSUM evacuated in [128,512] full-bank ACTIVATEs on the scalar engine;
    DVE does xpos muls/combines (bf16 2x mode) + the mask multiplies;
    gpsimd (SBUF-only engine) takes 3 of the 12 xpos tensor-tensor ops.
"""

import numpy as np

L = 243
H = 256
J = 17
NCORES = 8
NB = 1088
BPC = NB // NCORES          # 136 batch rows per core
NPAIR = BPC // 2            # 68 pairs per core
SCALE_BASE = 512
CHUNK = 81

_cache = {}


def _bf16():
    import ml_dtypes
    return ml_dtypes.bfloat16


def _host_tables(W_Q, W_K, W_V, gamma):
    f32 = np.float32
    half = H // 2

    # even/odd permutation of head-dim columns
    pe = np.arange(0, H, 2)
    po = np.arange(1, H, 2)
    Wq = W_Q.astype(f32)
    Wk = W_K.astype(f32)
    Wv = W_V.astype(f32)
    # cols: [Qe | Qo | Ke | Ko | V]  (256 x 768)
    Wcat = np.concatenate(
        [Wq[:, pe], Wq[:, po], Wk[:, pe], Wk[:, po], Wv], axis=1)
    W_all = np.stack([Wcat[0:128], Wcat[128:256]], axis=0)  # (2,128,768)

    # half-tables (128, L) -> packed pair layout (128, 512)
    base_scale = ((np.arange(0, H, 2, dtype=f32) + 0.4 * H) / (1.4 * H)).astype(f32)
    pos = np.arange(L, dtype=f32)
    scale = base_scale[None, :] ** (pos / SCALE_BASE)[:, None]        # (L, half)
    inv_freq = (1.0 / 10000.0 ** (np.arange(half, dtype=f32) / half)).astype(f32)
    sinus = pos[:, None] * inv_freq[None, :]
    sin, cos = np.sin(sinus).astype(f32), np.cos(sinus).astype(f32)
    hCq = (cos * scale).T
    hSq = (sin * scale).T
    hCk = (cos / scale).T
    hSk = (sin / scale).T

    CS = np.zeros((4, 128, 512), f32)
    for i, tbl in enumerate([hCq, hSq, hCk, hSk]):
        CS[i, :, 0:L] = tbl
        CS[i, :, 256:256 + L] = tbl

    # decay mask, transposed per joint, packed [m-tile0 | m-tile1] in cols:
    # DTP[j][m', 0:256)    = D[j, l, m']        (m' in [0,128))
    # DTP[j][m', 256:512)  = D[j, l, 128+m']    (m' in [0,115))
    g = gamma.astype(f32)
    i = np.arange(L)[:, None]
    jj = np.arange(L)[None, :]
    allowed = jj < (i // CHUNK + 1) * CHUNK
    absd = np.abs(i - jj).astype(f32)
    D = g[:, None, None] ** absd[None]
    D = np.where(allowed[None], D, 0.0)
    D = np.where(np.isnan(D), 0.0, D).astype(f32)
    DT = np.transpose(D, (0, 2, 1))                       # (J, m, l)
    DTP = np.zeros((J, 128, 512), f32)
    DTP[:, :, 0:L] = DT[:, 0:128, :]
    DTP[:, 0:L - 128, 256:256 + L] = DT[:, 128:L, :]
    return W_all, CS, DTP


def _host_pack_x(Xc):
    # (BPC, 243, 256) f32 -> (BPC, 256, 256) bf16, l-rows padded to 256
    Xp = np.zeros((BPC, 256, H), np.float32)
    Xp[:, :L] = Xc
    return np.ascontiguousarray(Xp.astype(_bf16()))


def _host_unpack_o(Oc):
    # (NPAIR, 128, 1024) -> (BPC, 243, 256)
    Op = Oc.reshape(NPAIR, 128, 2, 2, H)
    Op = np.transpose(Op, (0, 2, 3, 1, 4))        # t, b, lc, p, h
    Op = Op.reshape(BPC, 256, H)
    return np.ascontiguousarray(Op[:, :L])


def _build():
    import concourse.bacc as bacc
    import concourse.mybir as mybir
    from concourse import tile

    dt = mybir.dt
    f32 = dt.float32
    bf16 = dt.bfloat16

    nc = bacc.Bacc("TRN2", target_bir_lowering=False, debug=False,
                   num_devices=NCORES)
    X_d = nc.dram_tensor("X", (BPC, 256, H), bf16, kind="ExternalInput").ap()
    W_d = nc.dram_tensor("WALL", (2, 128, 768), f32, kind="ExternalInput").ap()
    CS_d = nc.dram_tensor("CS", (4, 128, 512), f32, kind="ExternalInput").ap()
    DT_d = nc.dram_tensor("DTAB", (J, 128, 512), f32, kind="ExternalInput").ap()
    O_d = nc.dram_tensor("OUT", (NPAIR, 128, 1024), f32, kind="ExternalOutput").ap()

    MSZ = (128, L - 128)          # m-tile sizes (128, 115)

    with tile.TileContext(nc) as tc:
        with (
            tc.tile_pool(name="const", bufs=1) as const,
            tc.tile_pool(name="work", bufs=2) as work,
            tc.tile_pool(name="pproj", bufs=2, space="PSUM") as pproj,
            tc.tile_pool(name="psv", bufs=6, space="PSUM") as psv,
        ):
            # ---- constants ----
            w_f32 = [const.tile([128, 768], f32, name=f"wf{h}", tag=f"wf{h}")
                     for h in range(2)]
            w_sb = [const.tile([128, 768], bf16, name=f"w{h}", tag=f"w{h}")
                    for h in range(2)]
            cs_f32 = const.tile([128, 2048], f32, name="csf", tag="csf")
            cs_sb = const.tile([128, 2048], bf16, name="cs", tag="cs")
            dt_f32 = [const.tile([128, 512], f32, name=f"dtf{j}", tag=f"dtf{j}")
                      for j in range(J)]
            dt_sb = [const.tile([128, 512], bf16, name=f"dt{j}", tag=f"dt{j}")
                     for j in range(J)]
            for h in range(2):
                nc.sync.dma_start(w_f32[h][:], W_d[h])
                nc.scalar.copy(w_sb[h][:], w_f32[h][:])
            for i in range(4):
                nc.sync.dma_start(cs_f32[:, i * 512:(i + 1) * 512], CS_d[i])
            nc.scalar.copy(cs_sb[:], cs_f32[:])
            for j in range(J):
                nc.sync.dma_start(dt_f32[j][:], DT_d[j])
                nc.scalar.copy(dt_sb[j][:], dt_f32[j][:])

            for t in range(NPAIR):
                b0 = 2 * t
                joints = (b0 % J, (b0 + 1) % J)

                # ---- XBAR DMA-transpose loads: XT with h on partitions ----
                # xt[hc] cols: k*256 + l (l-pads 243..255 zero from host pad)
                xt_sb = []
                for hc in range(2):
                    sb = work.tile([128, 512], bf16, name=f"xt{hc}", tag=f"xt{hc}")
                    for k in range(2):
                        nc.sync.dma_start(
                            sb[:, k * 256:k * 256 + 256],
                            X_d[b0 + k, :, hc * 128:hc * 128 + 128],
                            transpose=True)
                    xt_sb.append(sb)

                # ---- projections Qe,Qo,Ke,Ko (d-half on partitions, pair
                # packed along free), evacuated to bf16 as they finish ----
                pj_sb = []
                for i in range(4):
                    ps = pproj.tile([128, 512], f32, name="proj", tag="proj")
                    for hc in range(2):
                        nc.tensor.matmul(
                            ps[:],
                            w_sb[hc][:, i * 128:(i + 1) * 128],
                            xt_sb[hc][:],
                            start=(hc == 0), stop=(hc == 1),
                        )
                    sb = work.tile([128, 512], bf16, name=f"pj{i}", tag=f"pj{i}")
                    nc.scalar.copy(sb[:], ps[:])
                    pj_sb.append(sb)

                # ---- xpos combine (DVE bf16 2x; 3 muls on gpsimd) ----
                #  Qx_e = Qe*hC - Qo*hS ; Qx_o = Qo*hC + Qe*hS (K likewise)
                qk = []
                for ti in range(2):          # 0=Q, 1=K
                    pe_b, po_b = pj_sb[2 * ti], pj_sb[2 * ti + 1]
                    ctab = cs_sb[:, (2 * ti) * 512:(2 * ti) * 512 + 512]
                    stab = cs_sb[:, (2 * ti + 1) * 512:(2 * ti + 1) * 512 + 512]
                    t1 = work.tile([128, 512], bf16, name="t1", tag="t1")
                    t2 = work.tile([128, 512], bf16, name="t2", tag="t2")
                    t3 = work.tile([128, 512], bf16, name="t3", tag="t3")
                    t4 = work.tile([128, 512], bf16, name="t4", tag="t4")
                    xe = work.tile([128, 512], bf16, name=f"xe{ti}", tag=f"xe{ti}")
                    xo = work.tile([128, 512], bf16, name=f"xo{ti}", tag=f"xo{ti}")
                    nc.vector.tensor_mul(t1[:], pe_b[:], ctab)
                    nc.gpsimd.tensor_mul(t2[:], po_b[:], stab)
                    nc.vector.tensor_sub(xe[:], t1[:], t2[:])
                    if ti == 0:
                        nc.gpsimd.tensor_mul(t3[:], po_b[:], ctab)
                    else:
                        nc.vector.tensor_mul(t3[:], po_b[:], ctab)
                    nc.vector.tensor_mul(t4[:], pe_b[:], stab)
                    nc.vector.tensor_add(xo[:], t3[:], t4[:])
                    qk.append((xe, xo))
                (qx_e, qx_o), (kx_e, kx_o) = qk

                # ---- V = X @ Wv (natural layout; both m-tiles in one bank:
                # cols 0:256 = m in [0,128), cols 256:512 = m in [128,243)) ----
                v_sb = []
                for k in range(2):
                    ps = psv.tile([128, 512], f32, name="vps", tag="ps")
                    for mc in range(2):
                        msz = MSZ[mc]
                        for hc in range(2):
                            nc.tensor.matmul(
                                ps[0:msz, mc * 256:mc * 256 + 256],
                                xt_sb[hc][:, k * 256 + mc * 128:
                                          k * 256 + mc * 128 + msz],
                                w_sb[hc][:, 512:768],
                                start=(hc == 0), stop=(hc == 1),
                            )
                    sb = work.tile([128, 512], bf16, name=f"v{k}", tag=f"v{k}")
                    nc.scalar.copy(sb[:], ps[:])
                    v_sb.append(sb)

                # ---- attention ----
                ob = work.tile([128, 1024], f32, name="ob", tag="ob")
                for k in range(2):
                    jt = joints[k]
                    # scores S^T, both m-tiles in one bank; m-tile1 only needs
                    # l >= 81 (block-causal; dt table is 0 elsewhere)
                    ps = psv.tile([128, 512], f32, name="sps", tag="ps")
                    nc.tensor.matmul(
                        ps[0:128, 0:L],
                        kx_e[:, k * 256:k * 256 + 128],
                        qx_e[:, k * 256:k * 256 + L],
                        start=True, stop=False)
                    nc.tensor.matmul(
                        ps[0:128, 0:L],
                        kx_o[:, k * 256:k * 256 + 128],
                        qx_o[:, k * 256:k * 256 + L],
                        start=False, stop=True)
                    nc.tensor.matmul(
                        ps[0:MSZ[1], 256 + 81:256 + L],
                        kx_e[:, k * 256 + 128:k * 256 + 128 + MSZ[1]],
                        qx_e[:, k * 256 + 81:k * 256 + L],
                        start=True, stop=False)
                    nc.tensor.matmul(
                        ps[0:MSZ[1], 256 + 81:256 + L],
                        kx_o[:, k * 256 + 128:k * 256 + 128 + MSZ[1]],
                        qx_o[:, k * 256 + 81:k * 256 + L],
                        start=False, stop=True)
                    at = work.tile([128, 512], bf16, name=f"at{k}", tag=f"at{k}")
                    nc.vector.tensor_mul(at[:], ps[:], dt_sb[jt][:])

                    # out = A @ V
                    po = psv.tile([128, 512], f32, name="ops", tag="ps")
                    for lc in range(2):
                        lsz = MSZ[lc]
                        for mc in range(2):
                            nc.tensor.matmul(
                                po[0:lsz, lc * 256:lc * 256 + 256],
                                at[0:MSZ[mc], mc * 256 + lc * 128:
                                   mc * 256 + lc * 128 + lsz],
                                v_sb[k][0:MSZ[mc], mc * 256:mc * 256 + 256],
                                start=(mc == 0), stop=(mc == 1),
                            )
                    nc.scalar.copy(ob[:, k * 512:k * 512 + 512], po[:])

                nc.sync.dma_start(O_d[t], ob[:])

    nc.compile()
    return nc


def _get_nc():
    if "nc" not in _cache:
        _cache["nc"] = _build()
    return _cache["nc"]


def _run(in_maps, trace=False):
    from concourse import bass_utils
    nc = _get_nc()
    return bass_utils.run_bass_kernel_spmd(
        nc, in_maps, core_ids=list(range(NCORES)), trace=trace)


def kernel(X, W_Q, W_K, W_V, gamma, _trace=False):
    X = np.asarray(X, np.float32)
    W_all, CS, DTP = _host_tables(
        np.asarray(W_Q, np.float32), np.asarray(W_K, np.float32),
        np.asarray(W_V, np.float32), np.asarray(gamma, np.float32))

    in_maps = []
    for c in range(NCORES):
        in_maps.append({
            "X": _host_pack_x(X[c * BPC:(c + 1) * BPC]),
            "WALL": W_all, "CS": CS, "DTAB": DTP,
        })
    res = _run(in_maps, trace=_trace)
    out = np.concatenate([_host_unpack_o(r["OUT"]) for r in res.results],
                         axis=0)
    if _trace:
        _cache["last_result"] = res
    return out


# revision 11
# speedup vs baseline: 1.0754x; 1.0754x over previous
"""JointRetention Trainium2 kernel.

out[b] = ((xpos(X_b Wq) xpos_down(X_b Wk)^T) * D[b%17]) @ (X_b Wv)

Strategy (v4):
  - Data-parallel over B*J=1088 across 8 cores (136 each; 136%17==0 so the
    joint index pattern is identical on every core).
  - Even/odd d-permutation: with head-dim columns reordered [even | odd],
    rotate_every_two becomes a half-swap, so xpos needs NO extra projection:
      Qx_e = Qe*hC - Qo*hS ;  Qx_o = Qo*hC + Qe*hS
  - X stored bf16 as (pair, (b,l), h) so ONE XBAR DMA-transpose per h-half
    loads a whole pair's XT (h on partitions) - no PE transposes, no psum
    round trip for the input.
  - All matmul operands bf16 (f32 PSUM): 1 cycle/col at any free size.
  - Block-causal sparsity: scores for m in [128,243) computed only for
    l >= 81 (decay mask zeroes the rest).
  - PSUM evacuated in [128,512] full-bank ACTIVATEs on the scalar engine;
    DVE does xpos muls/combines (bf16 2x mode) + the mask multiplies;
    gpsimd (SBUF-only) takes 3 of the 12 xpos tensor-tensor ops.
  - Deep buffering (bufs=4 SBUF, 3+5 PSUM banks) + XBAR prefetch skew so
    the out-of-order tile scheduler can overlap several pairs.
"""

import numpy as np

L = 243
H = 256
J = 17
NCORES = 8
NB = 1088
BPC = NB // NCORES          # 136 batch rows per core
NPAIR = BPC // 2            # 68 pairs per core
SCALE_BASE = 512
CHUNK = 81

_cache = {}


def _bf16():
    import ml_dtypes
    return ml_dtypes.bfloat16


def _host_tables(W_Q, W_K, W_V, gamma):
    f32 = np.float32
    half = H // 2

    # even/odd permutation of head-dim columns
    pe = np.arange(0, H, 2)
    po = np.arange(1, H, 2)
    Wq = W_Q.astype(f32)
    Wk = W_K.astype(f32)
    Wv = W_V.astype(f32)
    # cols: [Qe | Qo | Ke | Ko | V]  (256 x 768)
    Wcat = np.concatenate(
        [Wq[:, pe], Wq[:, po], Wk[:, pe], Wk[:, po], Wv], axis=1)
    W_all = np.stack([Wcat[0:128], Wcat[128:256]], axis=0)  # (2,128,768)

    # half-tables (128, L) -> packed pair layout (128, 512)
    base_scale = ((np.arange(0, H, 2, dtype=f32) + 0.4 * H) / (1.4 * H)).astype(f32)
    pos = np.arange(L, dtype=f32)
    scale = base_scale[None, :] ** (pos / SCALE_BASE)[:, None]        # (L, half)
    inv_freq = (1.0 / 10000.0 ** (np.arange(half, dtype=f32) / half)).astype(f32)
    sinus = pos[:, None] * inv_freq[None, :]
    sin, cos = np.sin(sinus).astype(f32), np.cos(sinus).astype(f32)
    hCq = (cos * scale).T
    hSq = (sin * scale).T
    hCk = (cos / scale).T
    hSk = (sin / scale).T

    CS = np.zeros((4, 128, 512), f32)
    for i, tbl in enumerate([hCq, hSq, hCk, hSk]):
        CS[i, :, 0:L] = tbl
        CS[i, :, 256:256 + L] = tbl

    # decay mask, transposed per joint, packed [m-tile0 | m-tile1] in cols:
    # DTP[j][m', 0:256)    = D[j, l, m']        (m' in [0,128))
    # DTP[j][m', 256:512)  = D[j, l, 128+m']    (m' in [0,115))
    g = gamma.astype(f32)
    i = np.arange(L)[:, None]
    jj = np.arange(L)[None, :]
    allowed = jj < (i // CHUNK + 1) * CHUNK
    absd = np.abs(i - jj).astype(f32)
    D = g[:, None, None] ** absd[None]
    D = np.where(allowed[None], D, 0.0)
    D = np.where(np.isnan(D), 0.0, D).astype(f32)
    DT = np.transpose(D, (0, 2, 1))                       # (J, m, l)
    DTP = np.zeros((J, 128, 512), f32)
    DTP[:, :, 0:L] = DT[:, 0:128, :]
    DTP[:, 0:L - 128, 256:256 + L] = DT[:, 128:L, :]
    return W_all, CS, DTP.astype(_bf16())


def _host_pack_x(Xc):
    # (BPC, 243, 256) f32 -> (NPAIR, 512, 256) bf16: (t, (b,l), h) with
    # l-rows padded to 256
    Xp = np.zeros((BPC, 256, H), np.float32)
    Xp[:, :L] = Xc
    return np.ascontiguousarray(
        Xp.astype(_bf16()).reshape(NPAIR, 512, H))


def _host_unpack_o(Oc):
    # (NPAIR, 128, 1024) -> (BPC, 243, 256)
    Op = Oc.reshape(NPAIR, 128, 2, 2, H)
    Op = np.transpose(Op, (0, 2, 3, 1, 4))        # t, b, lc, p, h
    Op = Op.reshape(BPC, 256, H)
    return np.ascontiguousarray(Op[:, :L])


def _build():
    import concourse.bacc as bacc
    import concourse.mybir as mybir
    from concourse import tile

    dt = mybir.dt
    f32 = dt.float32
    bf16 = dt.bfloat16

    nc = bacc.Bacc("TRN2", target_bir_lowering=False, debug=False,
                   num_devices=NCORES)
    X_d = nc.dram_tensor("X", (NPAIR, 512, H), bf16, kind="ExternalInput").ap()
    W_d = nc.dram_tensor("WALL", (2, 128, 768), f32, kind="ExternalInput").ap()
    CS_d = nc.dram_tensor("CS", (4, 128, 512), f32, kind="ExternalInput").ap()
    DT_d = nc.dram_tensor("DTAB", (J, 128, 512), bf16, kind="ExternalInput").ap()
    O_d = nc.dram_tensor("OUT", (NPAIR, 128, 1024), f32, kind="ExternalOutput").ap()

    MSZ = (128, L - 128)          # m-tile sizes (128, 115)

    with tile.TileContext(nc) as tc:
        with (
            tc.tile_pool(name="const", bufs=1) as const,
            tc.tile_pool(name="work", bufs=4) as work,
            tc.tile_pool(name="pproj", bufs=3, space="PSUM") as pproj,
            tc.tile_pool(name="psv", bufs=5, space="PSUM") as psv,
        ):
            # ---- constants ----
            w_f32 = [const.tile([128, 768], f32, name=f"wf{h}", tag=f"wf{h}")
                     for h in range(2)]
            w_sb = [const.tile([128, 768], bf16, name=f"w{h}", tag=f"w{h}")
                    for h in range(2)]
            cs_f32 = const.tile([128, 2048], f32, name="csf", tag="csf")
            cs_sb = const.tile([128, 2048], bf16, name="cs", tag="cs")
            dt_sb = [const.tile([128, 512], bf16, name=f"dt{j}", tag=f"dt{j}")
                     for j in range(J)]
            for h in range(2):
                nc.sync.dma_start(w_f32[h][:], W_d[h])
                nc.scalar.copy(w_sb[h][:], w_f32[h][:])
            for i in range(4):
                nc.sync.dma_start(cs_f32[:, i * 512:(i + 1) * 512], CS_d[i])
            nc.scalar.copy(cs_sb[:], cs_f32[:])
            for j in range(J):
                nc.sync.dma_start(dt_sb[j][:], DT_d[j])

            def load_xt(t):
                # XBAR transposes: X_d[t] is (512=(b,l), 256=h); per (hc, k)
                # transpose [256, 128] -> [128, 256] into cols k*256+l.
                tiles = []
                for hc in range(2):
                    sb = work.tile([128, 512], bf16, name=f"xt{hc}",
                                   tag=f"xt{hc}")
                    for k in range(2):
                        nc.sync.dma_start(
                            sb[:, k * 256:k * 256 + 256],
                            X_d[t, k * 256:k * 256 + 256,
                                hc * 128:hc * 128 + 128],
                            transpose=True)
                    tiles.append(sb)
                return tiles

            xt_next = load_xt(0)

            for t in range(NPAIR):
                b0 = 2 * t
                joints = (b0 % J, (b0 + 1) % J)
                xt_sb = xt_next

                # ---- projections Qe,Qo,Ke,Ko (d-half on partitions, pair
                # packed along free), evacuated to bf16 as they finish ----
                pj_sb = []
                for i in range(4):
                    ps = pproj.tile([128, 512], f32, name="proj", tag="proj")
                    for hc in range(2):
                        nc.tensor.matmul(
                            ps[:],
                            w_sb[hc][:, i * 128:(i + 1) * 128],
                            xt_sb[hc][:],
                            start=(hc == 0), stop=(hc == 1),
                        )
                    sb = work.tile([128, 512], bf16, name=f"pj{i}", tag=f"pj{i}")
                    nc.scalar.copy(sb[:], ps[:])
                    pj_sb.append(sb)

                # ---- xpos combine (DVE bf16 2x; 3 muls on gpsimd) ----
                #  Qx_e = Qe*hC - Qo*hS ; Qx_o = Qo*hC + Qe*hS (K likewise)
                qk = []
                for ti in range(2):          # 0=Q, 1=K
                    pe_b, po_b = pj_sb[2 * ti], pj_sb[2 * ti + 1]
                    ctab = cs_sb[:, (2 * ti) * 512:(2 * ti) * 512 + 512]
                    stab = cs_sb[:, (2 * ti + 1) * 512:(2 * ti + 1) * 512 + 512]
                    t1 = work.tile([128, 512], bf16, name="t1", tag="t1")
                    t2 = work.tile([128, 512], bf16, name="t2", tag="t2")
                    t3 = work.tile([128, 512], bf16, name="t3", tag="t3")
                    t4 = work.tile([128, 512], bf16, name="t4", tag="t4")
                    xe = work.tile([128, 512], bf16, name=f"xe{ti}", tag=f"xe{ti}")
                    xo = work.tile([128, 512], bf16, name=f"xo{ti}", tag=f"xo{ti}")
                    nc.vector.tensor_mul(t1[:], pe_b[:], ctab)
                    nc.gpsimd.tensor_mul(t2[:], po_b[:], stab)
                    nc.vector.tensor_sub(xe[:], t1[:], t2[:])
                    if ti == 0:
                        nc.gpsimd.tensor_mul(t3[:], po_b[:], ctab)
                    else:
                        nc.vector.tensor_mul(t3[:], po_b[:], ctab)
                    nc.vector.tensor_mul(t4[:], pe_b[:], stab)
                    nc.vector.tensor_add(xo[:], t3[:], t4[:])
                    qk.append((xe, xo))
                (qx_e, qx_o), (kx_e, kx_o) = qk

                # ---- prefetch next pair's XT (independent of this pair) ----
                if t + 1 < NPAIR:
                    xt_next = load_xt(t + 1)

                # ---- V = X @ Wv (natural layout; both m-tiles in one bank:
                # cols 0:256 = m in [0,128), cols 256:512 = m in [128,243)) ----
                v_sb = []
                for k in range(2):
                    ps = psv.tile([128, 512], f32, name="vps", tag="ps")
                    for mc in range(2):
                        msz = MSZ[mc]
                        for hc in range(2):
                            nc.tensor.matmul(
                                ps[0:msz, mc * 256:mc * 256 + 256],
                                xt_sb[hc][:, k * 256 + mc * 128:
                                          k * 256 + mc * 128 + msz],
                                w_sb[hc][:, 512:768],
                                start=(hc == 0), stop=(hc == 1),
                            )
                    sb = work.tile([128, 512], bf16, name=f"v{k}", tag=f"v{k}")
                    nc.scalar.copy(sb[:], ps[:])
                    v_sb.append(sb)

                # ---- attention ----
                ob = work.tile([128, 1024], f32, name="ob", tag="ob")
                for k in range(2):
                    jt = joints[k]
                    # scores S^T, both m-tiles in one bank; m-tile1 only needs
                    # l >= 81 (block-causal; dt table is 0 elsewhere)
                    ps = psv.tile([128, 512], f32, name="sps", tag="ps")
                    nc.tensor.matmul(
                        ps[0:128, 0:L],
                        kx_e[:, k * 256:k * 256 + 128],
                        qx_e[:, k * 256:k * 256 + L],
                        start=True, stop=False)
                    nc.tensor.matmul(
                        ps[0:128, 0:L],
                        kx_o[:, k * 256:k * 256 + 128],
                        qx_o[:, k * 256:k * 256 + L],
                        start=False, stop=True)
                    nc.tensor.matmul(
                        ps[0:MSZ[1], 256 + 81:256 + L],
                        kx_e[:, k * 256 + 128:k * 256 + 128 + MSZ[1]],
                        qx_e[:, k * 256 + 81:k * 256 + L],
                        start=True, stop=False)
                    nc.tensor.matmul(
                        ps[0:MSZ[1], 256 + 81:256 + L],
                        kx_o[:, k * 256 + 128:k * 256 + 128 + MSZ[1]],
                        qx_o[:, k * 256 + 81:k * 256 + L],
                        start=False, stop=True)
                    at = work.tile([128, 512], bf16, name=f"at{k}", tag=f"at{k}")
                    nc.vector.tensor_mul(at[:], ps[:], dt_sb[jt][:])

                    # out = A @ V
                    po = psv.tile([128, 512], f32, name="ops", tag="ps")
                    for lc in range(2):
                        lsz = MSZ[lc]
                        for mc in range(2):
                            nc.tensor.matmul(
                                po[0:lsz, lc * 256:lc * 256 + 256],
                                at[0:MSZ[mc], mc * 256 + lc * 128:
                                   mc * 256 + lc * 128 + lsz],
                                v_sb[k][0:MSZ[mc], mc * 256:mc * 256 + 256],
                                start=(mc == 0), stop=(mc == 1),
                            )
                    nc.scalar.copy(ob[:, k * 512:k * 512 + 512], po[:])

                nc.sync.dma_start(O_d[t], ob[:])

    nc.compile()
    return nc


def _get_nc():
    if "nc" not in _cache:
        _cache["nc"] = _build()
    return _cache["nc"]


def _run(in_maps, trace=False):
    from concourse import bass_utils
    nc = _get_nc()
    return bass_utils.run_bass_kernel_spmd(
        nc, in_maps, core_ids=list(range(NCORES)), trace=trace)


def kernel(X, W_Q, W_K, W_V, gamma, _trace=False):
    X = np.asarray(X, np.float32)
    W_all, CS, DTP = _host_tables(
        np.asarray(W_Q, np.float32), np.asarray(W_K, np.float32),
        np.asarray(W_V, np.float32), np.asarray(gamma, np.float32))

    in_maps = []
    for c in range(NCORES):
        in_maps.append({
            "X": _host_pack_x(X[c * BPC:(c + 1) * BPC]),
            "WALL": W_all, "CS": CS, "DTAB": DTP,
        })
    res = _run(in_maps, trace=_trace)
    out = np.concatenate([_host_unpack_o(r["OUT"]) for r in res.results],
                         axis=0)
    if _trace:
        _cache["last_result"] = res
    return out
